# revision 1
# baseline (speedup 1.0000x reference)
"""ConstituencyTreeLSTM Trainium2 kernel.

Strategy:
  - Data-parallel over the B=256 batch across 8 NeuronCores (32 rows/core).
  - The tree is a complete heap (node i has children 2i+1, 2i+2), so the
    sequential scan is reorganized into level-parallel phases:
      leaves (nodes 128..255) -> node 127 -> level 6 (63..126) -> ... -> root.
  - Everything on-device lives in a "feature-on-partitions, (node,batch) rows
    on free axis" layout, so matmul outputs (PSUM, [out_dim, rows]) are already
    in the layout needed to feed the next level's matmul. No transposes.
  - One fused bf16 weight matrix W_big [1536, 2560]:
      rows:  0:512 x | 512:1024 hL | 1024:1536 hR
      cols:  0:1536 iou | 1536:2048 fL-pre | 2048:2560 fR-pre
    Zero blocks (hR->fL, hL->fR) are skipped; only the 208 used 128x128
    blocks are stored (packed).
  - h of every level lives in SBUF level tiles; parents read children h via
    stride-2 node slices directly (no DRAM roundtrip on the critical path).
  - c goes through DRAM (CL/CR, parity-split by parent index) - it is only
    needed by the cheap elementwise stage, late in each chunk.
  - Per-node-type biases (2-child / leaf / 1-child) folded host-side and
    applied inside the PSUM-evacuating activation (sigmoid/tanh).
"""

import sys

sys.path.insert(0, "/opt/trn_rl_repo")

import numpy as np
import ml_dtypes

import concourse.bass as bass  # noqa: F401
import concourse.mybir as mybir
import concourse.tile as tile
from concourse import bacc
from concourse.bass_utils import run_bass_kernel_spmd

BF16 = ml_dtypes.bfloat16
NCORES = 8
B, N, D = 256, 256, 512
BC = B // NCORES  # batch rows per core
KT_X, KT_HL, KT_HR = range(0, 4), range(4, 8), range(8, 12)
NJ = 20  # output j-tiles: 12 iou + 4 fL + 4 fR

_compiled = {}


def _used_kts(j, has_l=True, has_r=True):
    if j < 12:
        kts = list(KT_X) + (list(KT_HL) if has_l else []) + (list(KT_HR) if has_r else [])
    elif j < 16:
        kts = list(KT_X) + list(KT_HL)
    else:
        kts = list(KT_X) + list(KT_HR)
    return kts


# packed weight-block index: only (kt, j) pairs with nonzero weight blocks
W_BLOCKS = [(kt, j) for j in range(NJ) for kt in _used_kts(j)]
W_IDX = {p: i for i, p in enumerate(W_BLOCKS)}
NW = len(W_BLOCKS)  # 208


def _build_bass(reps=1, kts_limit=None, skip_ew=False):
    nc = bacc.Bacc("TRN2", target_bir_lowering=False, debug=False, num_devices=NCORES)

    f32 = mybir.dt.float32
    bf16 = mybir.dt.bfloat16

    xt = nc.dram_tensor("xt", [N, D, BC], bf16, kind="ExternalInput")
    w = nc.dram_tensor("w", [NW, 128, 128], bf16, kind="ExternalInput")
    b2_d = nc.dram_tensor("b2", [128, NJ], f32, kind="ExternalInput")
    bleaf_d = nc.dram_tensor("bleaf", [128, NJ], f32, kind="ExternalInput")
    b1_d = nc.dram_tensor("b1", [128, NJ], f32, kind="ExternalInput")

    # children c keyed by parent index t: CL[t] = c(2t+1), CR[t] = c(2t+2)
    CL = nc.dram_tensor("CLbuf", [128, D, BC], bf16)
    CR = nc.dram_tensor("CRbuf", [128, D, BC], bf16)

    c0t = nc.dram_tensor("c0t", [D, BC], f32, kind="ExternalOutput")
    h0t = nc.dram_tensor("h0t", [D, BC], f32, kind="ExternalOutput")

    # all views are [partition, node, ktile, batch]; (node, ktile) merge on DMA
    xt_r = xt.ap().rearrange("n (kt p) b -> p n kt b", p=128)
    CL_r = CL.ap().rearrange("t (kt p) b -> p t kt b", p=128)
    CR_r = CR.ap().rearrange("t (kt p) b -> p t kt b", p=128)
    c0t_r = c0t.ap().rearrange("(kt p) b -> p kt b", p=128)
    h0t_r = h0t.ap().rearrange("(kt p) b -> p kt b", p=128)

    with tile.TileContext(nc) as tc:
        import contextlib

        ctx = contextlib.ExitStack()
        with ctx:
            wpool = ctx.enter_context(tc.tile_pool(name="wpool", bufs=1))
            hpool = ctx.enter_context(tc.tile_pool(name="hpool", bufs=1))
            inpool = ctx.enter_context(tc.tile_pool(name="inpool", bufs=2))
            gpool = ctx.enter_context(tc.tile_pool(name="gpool", bufs=2))
            epool = ctx.enter_context(tc.tile_pool(name="epool", bufs=2))
            pspool = ctx.enter_context(tc.tile_pool(name="ps", bufs=8, space="PSUM"))

            w_sb = wpool.tile([128, NW, 128], bf16)
            nc.sync.dma_start(out=w_sb[:], in_=w.ap().rearrange("blk p c -> p blk c"))
            b2_sb = wpool.tile([128, NJ], f32, name="b2sb")
            bleaf_sb = wpool.tile([128, NJ], f32, name="bleafsb")
            b1_sb = wpool.tile([128, NJ], f32, name="b1sb")
            nc.sync.dma_start(out=b2_sb[:], in_=b2_d.ap()[:])
            nc.sync.dma_start(out=bleaf_sb[:], in_=bleaf_d.ap()[:])
            nc.sync.dma_start(out=b1_sb[:], in_=b1_d.ap()[:])

            def process(
                nodes,
                has_l,
                has_r,
                bias_sb,
                child_h,  # (tile, base_node) or None
                out_h,  # (tile, base_node) or None (root)
                child_c=None,  # (tile, base_node) -> read children c from SBUF
                out_c=None,  # (tile, base_node) -> write c to SBUF, skip CL/CR
            ):
                """Compute (c, h) for `nodes` (a range), all at the same depth."""
                to_out = out_h is None
                for a in range(nodes.start, nodes.stop, 16):
                    b_ = min(a + 16, nodes.stop)
                    k = b_ - a  # nodes in this chunk
                    dt_g = f32 if to_out else bf16

                    xt_t = inpool.tile([128, k, 4, BC], bf16, name="xt_t")
                    nc.sync.dma_start(out=xt_t[:], in_=xt_r[:, a:b_, :, :])
                    if child_c is None:
                        if has_l:
                            cl_t = inpool.tile([128, k, 4, BC], bf16, name="cl_t")
                            nc.sync.dma_start(out=cl_t[:], in_=CL_r[:, a:b_, :, :])
                        if has_r:
                            cr_t = inpool.tile([128, k, 4, BC], bf16, name="cr_t")
                            nc.sync.dma_start(out=cr_t[:], in_=CR_r[:, a:b_, :, :])
                    if child_h is not None:
                        ch_t, ch_base = child_h
                        sl0 = 2 * a + 1 - ch_base

                        def child_slice(kt, off):
                            s0 = sl0 + off
                            if k == 1:
                                return ch_t[:, s0 : s0 + 1, kt, :]
                            return ch_t[:, s0 : s0 + 2 * k - 1 : 2, kt, :]

                    if child_c is not None:
                        cc_t, cc_base = child_c
                        cs0 = 2 * a + 1 - cc_base
                        if k == 1:
                            cl_t = cc_t[:, cs0 : cs0 + 1, :, :]
                            cr_t = cc_t[:, cs0 + 1 : cs0 + 2, :, :]
                        else:
                            cl_t = cc_t[:, cs0 : cs0 + 2 * k - 1 : 2, :, :]
                            cr_t = cc_t[:, cs0 + 1 : cs0 + 2 * k : 2, :, :]

                    g_i = gpool.tile([128, k, 4, BC], dt_g, name="g_i")
                    g_o = gpool.tile([128, k, 4, BC], dt_g, name="g_o")
                    g_u = gpool.tile([128, k, 4, BC], dt_g, name="g_u")
                    if has_l:
                        g_fl = gpool.tile([128, k, 4, BC], dt_g, name="g_fl", bufs=1)
                    if has_r:
                        g_fr = gpool.tile([128, k, 4, BC], dt_g, name="g_fr", bufs=1)

                    js = list(range(12))
                    if has_l:
                        js += list(range(12, 16))
                    if has_r:
                        js += list(range(16, 20))

                    for j in js:
                        kts = _used_kts(j, has_l, has_r)
                        if kts_limit:
                            kts = kts[:kts_limit]

                        ps = pspool.tile([128, k, BC], f32, name="ps")
                        for i, kt in enumerate(kts):
                            if kt < 4:
                                rhs = xt_t[:, :, kt, :]
                            elif kt < 8:
                                rhs = child_slice(kt - 4, 0)
                            else:
                                rhs = child_slice(kt - 8, 1)
                            nc.tensor.matmul(
                                ps[:],
                                w_sb[:, W_IDX[(kt, j)], :],
                                rhs,
                                start=(i == 0),
                                stop=(i == len(kts) - 1),
                            )
                        func = (
                            mybir.ActivationFunctionType.Tanh
                            if 8 <= j < 12
                            else mybir.ActivationFunctionType.Sigmoid
                        )
                        if j < 4:
                            dst = g_i[:, :, j, :]
                        elif j < 8:
                            dst = g_o[:, :, j - 4, :]
                        elif j < 12:
                            dst = g_u[:, :, j - 8, :]
                        elif j < 16:
                            dst = g_fl[:, :, j - 12, :]
                        else:
                            dst = g_fr[:, :, j - 16, :]
                        nc.scalar.activation(
                            out=dst,
                            in_=ps[:],
                            func=func,
                            bias=bias_sb[:, j : j + 1],
                            scale=1.0,
                        )

                    if skip_ew:
                        continue

                    # c = i*u (+ fl*cl) (+ fr*cr);  h = o * tanh(c)
                    if out_c is not None:
                        oc_t, oc_base = out_c
                        c_t = oc_t[:, a - oc_base : b_ - oc_base, :, :]
                    else:
                        c_t = epool.tile([128, k, 4, BC], dt_g, name="c_t")
                    nc.vector.tensor_mul(c_t[:], g_i[:], g_u[:])
                    if has_l:
                        m2 = epool.tile([128, k, 4, BC], dt_g, name="mt")
                        nc.vector.tensor_mul(m2[:], g_fl[:], cl_t[:])
                        nc.vector.tensor_add(c_t[:], c_t[:], m2[:])
                    if has_r:
                        m3 = epool.tile([128, k, 4, BC], dt_g, name="mt")
                        nc.vector.tensor_mul(m3[:], g_fr[:], cr_t[:])
                        nc.vector.tensor_add(c_t[:], c_t[:], m3[:])
                    tc_t = epool.tile([128, k, 4, BC], dt_g, name="tc_t")
                    nc.scalar.activation(
                        out=tc_t[:], in_=c_t[:], func=mybir.ActivationFunctionType.Tanh
                    )

                    if to_out:
                        h_t = epool.tile([128, k, 4, BC], dt_g, name="h_t")
                        nc.vector.tensor_mul(h_t[:], g_o[:], tc_t[:])
                        nc.sync.dma_start(out=c0t_r[:], in_=c_t[:, 0, :, :])
                        nc.sync.dma_start(out=h0t_r[:], in_=h_t[:, 0, :, :])
                    else:
                        oh_t, oh_base = out_h
                        nc.vector.tensor_mul(
                            oh_t[:, a - oh_base : b_ - oh_base, :, :], g_o[:], tc_t[:]
                        )
                        if out_c is not None:
                            continue  # c already written to its SBUF level tile
                        # c of node t -> CL[(t-1)//2] if t odd else CR[t//2 - 1]
                        odd0 = 0 if a % 2 == 1 else 1
                        even0 = 1 - odd0
                        odds = range(a + odd0, b_, 2)
                        evens = range(a + even0, b_, 2)
                        for kt in range(4):
                            if len(odds):
                                lo = (odds[0] - 1) // 2
                                nc.sync.dma_start(
                                    out=CL_r[:, lo : lo + len(odds), kt, :],
                                    in_=c_t[:, odd0::2, kt, :],
                                )
                            if len(evens):
                                ro = evens[0] // 2 - 1
                                nc.sync.dma_start(
                                    out=CR_r[:, ro : ro + len(evens), kt, :],
                                    in_=c_t[:, even0::2, kt, :],
                                )

            # c stays in SBUF for the small tail levels (outputs of L4..L1);
            # their parent phases then skip the CL/CR DRAM roundtrip entirely.
            C_SBUF_LVLS = (4, 3, 2, 1)

            for _rep in range(reps):
                # per-level h tiles (SBUF-resident)
                leafc_h = hpool.tile([128, 129, 4, BC], bf16, name="h_leafc")
                lvl_h = {7: (leafc_h, 127)}
                lvl_c = {}
                for lvl in range(6, 0, -1):
                    t = hpool.tile([128, 2**lvl, 4, BC], bf16, name=f"h_{lvl}")
                    lvl_h[lvl] = (t, 2**lvl - 1)
                for lvl in C_SBUF_LVLS:
                    t = hpool.tile([128, 2**lvl, 4, BC], bf16, name=f"c_{lvl}")
                    lvl_c[lvl] = (t, 2**lvl - 1)

                # leaves: nodes 128..255 (no children)
                process(range(128, 256), False, False, bleaf_sb, None, lvl_h[7])
                # node 127: left child only (node 255, leafc slot 128)
                process(range(127, 128), True, False, b1_sb, lvl_h[7], lvl_h[7])
                # levels 6..1: two children each
                for lvl in range(6, 0, -1):
                    process(
                        range(2**lvl - 1, 2 ** (lvl + 1) - 1),
                        True,
                        True,
                        b2_sb,
                        lvl_h[lvl + 1] if lvl < 6 else lvl_h[7],
                        lvl_h[lvl],
                        child_c=lvl_c.get(lvl + 1),
                        out_c=lvl_c.get(lvl),
                    )
                # root
                process(range(0, 1), True, True, b2_sb, lvl_h[1], None, child_c=lvl_c.get(1))

    nc.compile()
    return nc


def _expected_tree():
    left = np.array([2 * i + 1 if 2 * i + 1 < N else 0 for i in range(N)], np.int32)
    right = np.array([2 * i + 2 if 2 * i + 2 < N else 0 for i in range(N)], np.int32)
    nch = np.array(
        [int(2 * i + 1 < N) + int(2 * i + 2 < N) for i in range(N)], np.int32
    )
    return left, right, nch


def pack_w(W_ioux, W_fx, W_iouhL, W_fhL, W_iouhR, W_fhR):
    w_big = np.zeros((1536, 2560), np.float32)
    w_big[0:512, 0:1536] = np.asarray(W_ioux, np.float32).T
    w_big[0:512, 1536:2048] = np.asarray(W_fx, np.float32).T
    w_big[0:512, 2048:2560] = np.asarray(W_fx, np.float32).T
    w_big[512:1024, 0:1536] = np.asarray(W_iouhL, np.float32).T
    w_big[512:1024, 1536:2048] = np.asarray(W_fhL, np.float32).T
    w_big[1024:1536, 0:1536] = np.asarray(W_iouhR, np.float32).T
    w_big[1024:1536, 2048:2560] = np.asarray(W_fhR, np.float32).T
    w_np = np.empty((NW, 128, 128), np.float32)
    for i, (kt, j) in enumerate(W_BLOCKS):
        w_np[i] = w_big[kt * 128 : (kt + 1) * 128, j * 128 : (j + 1) * 128]
    return np.ascontiguousarray(w_np).astype(BF16)


def pack_biases(b_ioux, b_iouh, b_iouhL, b_iouhR, b_fx, b_fhL, b_fhR):
    def pack(vec):
        return np.ascontiguousarray(np.asarray(vec, np.float32).reshape(NJ, 128).T)

    z = np.zeros(512, np.float32)
    b2 = pack(np.concatenate([b_ioux + b_iouhL + b_iouhR, b_fx + b_fhL, b_fx + b_fhR]))
    bleaf = pack(np.concatenate([b_ioux + b_iouh, z, z]))
    b1 = pack(np.concatenate([b_ioux + b_iouhL, b_fx + b_fhL, z]))
    return b2, bleaf, b1


def kernel(
    inputs,
    W_ioux, b_ioux, W_iouh, b_iouh, W_iouhL, b_iouhL, W_iouhR, b_iouhR,
    W_fx, b_fx, W_fh, b_fh, W_fhL, b_fhL, W_fhR, b_fhR,
    left_idx, right_idx, num_children,
):
    el, er, en = _expected_tree()
    assert np.array_equal(np.asarray(left_idx), el), "unexpected tree structure"
    assert np.array_equal(np.asarray(right_idx), er), "unexpected tree structure"
    assert np.array_equal(np.asarray(num_children), en), "unexpected tree structure"

    inputs = np.asarray(inputs, np.float32)

    w_np = pack_w(W_ioux, W_fx, W_iouhL, W_fhL, W_iouhR, W_fhR)
    b_args = [
        np.asarray(v, np.float32)
        for v in (b_ioux, b_iouh, b_iouhL, b_iouhR, b_fx, b_fhL, b_fhR)
    ]
    b2, bleaf, b1 = pack_biases(*b_args)

    if "nc" not in _compiled:
        _compiled["nc"] = _build_bass()
    nc = _compiled["nc"]

    in_maps = []
    for c in range(NCORES):
        xc = inputs[c * BC : (c + 1) * BC]  # [BC, N, D]
        xt_c = np.ascontiguousarray(xc.transpose(1, 2, 0)).astype(BF16)  # [N, D, BC]
        in_maps.append({"xt": xt_c, "w": w_np, "b2": b2, "bleaf": bleaf, "b1": b1})

    res = run_bass_kernel_spmd(
        nc, in_maps, core_ids=list(range(NCORES)), trace=bool(_compiled.get("trace"))
    )
    _compiled["last_res"] = res

    c_full = np.empty((B, D), np.float32)
    h_full = np.empty((B, D), np.float32)
    for c in range(NCORES):
        c_full[c * BC : (c + 1) * BC] = res.results[c]["c0t"].T
        h_full[c * BC : (c + 1) * BC] = res.results[c]["h0t"].T
    return c_full, h_full



# revision 34
# speedup vs baseline: 1.4052x; 1.4052x over previous
"""ConstituencyTreeLSTM Trainium2 kernel.

Strategy:
  - Data-parallel over the B=256 batch across 8 NeuronCores (32 rows/core).
  - The tree is a complete heap (node i has children 2i+1, 2i+2), so the
    sequential scan is reorganized into level-parallel phases:
      leaves (nodes 128..255) -> node 127 -> level 6 (63..126) -> ... -> root.
  - Everything on-device lives in a "feature-on-partitions, (node, ktile,
    batch) on free axis" layout, so matmul outputs (PSUM, [out_dim, rows])
    feed the next level's matmuls with no transposes.
  - h-path matmuls at deep levels (node level >= 3) run in fp8e4m3 with
    DoubleRow perf mode (2 k-tiles per instruction, 2x MAC throughput);
    shallow levels (4+2+1 nodes) stay bf16 for accuracy. x-path matmuls
    are bf16 everywhere (fp8 x fails the error budget). All weights are
    pre-scaled by 16 (exact in bf16, keeps the fp8 h-weights out of the
    e4m3 denormal range); the PSUM-evacuating activation applies
    scale=1/16.
  - The f-gate x-projection (x @ W_fx) is computed once per chunk into
    fx_t (PSUM -> Copy-activation); fL/fR accumulate only their h-path in
    PSUM and a DVE add applies fx_t, removing a duplicated 16-matmul
    group per 2-child chunk.
  - h of every level lives in SBUF level tiles (fp8 for levels 4..7, bf16
    for 1..3); parents read children h via stride-2 node slices
    (rearranged to [p, ktpair, node, batch] for DoubleRow).
  - c goes through DRAM (CL/CR, parity-split by parent index) for the big
    levels; SBUF level tiles for levels 4..1.
  - Weight/bias DMAs ride the Activation HWDGE queue so the first xt tile
    (SP queue) isn't stuck behind them; leaves only wait for the 1.5MB
    iou x-weight tile instead of all weights.
"""

import sys

sys.path.insert(0, "/opt/trn_rl_repo")

import numpy as np
import ml_dtypes

import concourse.bass as bass  # noqa: F401
import concourse.mybir as mybir
import concourse.tile as tile
from concourse import bacc
from concourse.bass_utils import run_bass_kernel_spmd

BF16 = ml_dtypes.bfloat16
FP8 = ml_dtypes.float8_e4m3
NCORES = 8
B, N, D = 256, 256, 512
BC = B // NCORES  # batch rows per core
NJ = 20  # 12 iou + 4 fL + 4 fR bias columns
WSCALE = 16.0

# x-path blocks: 12 iou j-tiles + 4 fx j-tiles, 4 k-tiles each (bf16)
W_X_BLOCKS = [(kt, j) for j in range(16) for kt in range(4)]
WX_IDX = {p: i for i, p in enumerate(W_X_BLOCKS)}
NWX = len(W_X_BLOCKS)  # 64
NWX_IOU = 48  # first 12 js are the iou blocks the leaf phase needs

# h-path blocks, DoubleRow-pair adjacent: per iou j: hL kt 0..4 then hR kt
# 0..4; per fL j: hL kt 0..4; per fR j: hR kt 0..4
W_H_BLOCKS = []
for j in range(12):
    W_H_BLOCKS += [("L", kt, j) for kt in range(4)]
    W_H_BLOCKS += [("R", kt, j) for kt in range(4)]
for j in range(12, 16):
    W_H_BLOCKS += [("L", kt, j) for kt in range(4)]
for j in range(16, 20):
    W_H_BLOCKS += [("R", kt, j) for kt in range(4)]
WH_IDX = {p: i for i, p in enumerate(W_H_BLOCKS)}
NWH = len(W_H_BLOCKS)  # 128

_compiled = {}


def _build_bass(reps=1):
    nc = bacc.Bacc("TRN2", target_bir_lowering=False, debug=False, num_devices=NCORES)

    f32 = mybir.dt.float32
    bf16 = mybir.dt.bfloat16
    fp8 = mybir.dt.float8e4
    DR = mybir.MatmulPerfMode.DoubleRow
    ACT = mybir.ActivationFunctionType

    xt = nc.dram_tensor("xt", [N, D, BC], bf16, kind="ExternalInput")
    wx_d = nc.dram_tensor("wx", [NWX, 128, 128], bf16, kind="ExternalInput")
    wh8_d = nc.dram_tensor("wh8", [NWH, 128, 128], fp8, kind="ExternalInput")
    b2_d = nc.dram_tensor("b2", [128, NJ], f32, kind="ExternalInput")
    bleaf_d = nc.dram_tensor("bleaf", [128, NJ], f32, kind="ExternalInput")
    b1_d = nc.dram_tensor("b1", [128, NJ], f32, kind="ExternalInput")

    c0t = nc.dram_tensor("c0t", [D, BC], f32, kind="ExternalOutput")
    h0t = nc.dram_tensor("h0t", [D, BC], f32, kind="ExternalOutput")

    # views: [partition, node, ktile, batch]
    xt_r = xt.ap().rearrange("n (kt p) b -> p n kt b", p=128)
    c0t_r = c0t.ap().rearrange("(kt p) b -> p kt b", p=128)
    h0t_r = h0t.ap().rearrange("(kt p) b -> p kt b", p=128)

    with tile.TileContext(nc) as tc:
        import contextlib

        ctx = contextlib.ExitStack()
        with ctx:
            wpool = ctx.enter_context(tc.tile_pool(name="wpool", bufs=1))
            hpool = ctx.enter_context(tc.tile_pool(name="hpool", bufs=1))
            inpool = ctx.enter_context(tc.tile_pool(name="inpool", bufs=2))
            gpool = ctx.enter_context(tc.tile_pool(name="gpool", bufs=2))
            epool = ctx.enter_context(tc.tile_pool(name="epool", bufs=2))
            pspool = ctx.enter_context(tc.tile_pool(name="ps", bufs=8, space="PSUM"))

            # --- weights / biases ---------------------------------------
            # All weight DMAs ride the Pool (gpsimd) SWDGE queue in 16-block
            # pieces: small pieces interleave with the SP-queue xt prefetches
            # on the DMA engines instead of starving them, and the idle Pool
            # sequencer absorbs the issue cost. The leaf phase only needs the
            # wx_iou pieces (first on the queue) + bleaf (SP, tiny).
            wx_iou_sb = wpool.tile([128, NWX_IOU, 128], bf16, name="wxiou")
            wx_f_sb = wpool.tile([128, NWX - NWX_IOU, 128], bf16, name="wxf")
            wh8_sb = wpool.tile([128, NWH, 128], fp8, name="wh8")
            b2_sb = wpool.tile([128, NJ], f32, name="b2sb")
            bleaf_sb = wpool.tile([128, NJ], f32, name="bleafsb")
            b1_sb = wpool.tile([128, NJ], f32, name="b1sb")

            wx_r = wx_d.ap().rearrange("blk p c -> p blk c")
            wh8_r = wh8_d.ap().rearrange("blk p c -> p blk c")
            nc.sync.dma_start(out=bleaf_sb[:], in_=bleaf_d.ap()[:])
            for s in range(0, NWX_IOU, 16):
                nc.gpsimd.dma_start(
                    out=wx_iou_sb[:, s : s + 16, :], in_=wx_r[:, s : s + 16, :]
                )
            nc.gpsimd.dma_start(out=wx_f_sb[:], in_=wx_r[:, NWX_IOU:, :])
            for s in range(0, NWH, 16):
                nc.gpsimd.dma_start(
                    out=wh8_sb[:, s : s + 16, :], in_=wh8_r[:, s : s + 16, :]
                )
            nc.gpsimd.dma_start(out=b2_sb[:], in_=b2_d.ap()[:])
            nc.gpsimd.dma_start(out=b1_sb[:], in_=b1_d.ap()[:])

            def wx_ap(kt, j):
                if j < 12:
                    return wx_iou_sb[:, WX_IDX[(kt, j)], :]
                return wx_f_sb[:, WX_IDX[(kt, j)] - NWX_IOU, :]

            def process(
                nodes,
                has_l,
                has_r,
                bias_sb,
                child_h,  # list[(tile, base)] — 1 (plain fp8 h) or 2 (h8+res)
                out_h,  # list[(tile, base)] or None (root)
                child_c=None,  # (tile, base_node) -> children c from SBUF
                out_c=None,  # (tile, base_node) -> write c to SBUF
                chunk_starts=None,  # custom chunk order (e.g. L6 defers 63..78)
            ):
                """Compute (c, h) for `nodes` (a range), all at one depth."""
                to_out = out_h is None
                for a in chunk_starts or range(nodes.start, nodes.stop, 16):
                    b_ = min(a + 16, nodes.stop)
                    k = b_ - a  # nodes in this chunk
                    dt_g = f32 if to_out else bf16

                    xt_t = inpool.tile([128, k, 4, BC], bf16, name="xt_t")
                    nc.sync.dma_start(out=xt_t[:], in_=xt_r[:, a:b_, :, :])
                    if child_c is not None:
                        cc_t, cc_base = child_c
                        cs0 = 2 * a + 1 - cc_base
                        if has_l:
                            if k == 1:
                                cl_t = cc_t[:, cs0 : cs0 + 1, :, :]
                            else:
                                cl_t = cc_t[:, cs0 : cs0 + 2 * k - 1 : 2, :, :]
                        if has_r:
                            if k == 1:
                                cr_t = cc_t[:, cs0 + 1 : cs0 + 2, :, :]
                            else:
                                cr_t = cc_t[:, cs0 + 1 : cs0 + 2 * k : 2, :, :]

                    if child_h is not None:
                        ch_base = child_h[0][1]
                        sl0 = 2 * a + 1 - ch_base

                        def nsl(off):
                            s0 = sl0 + off
                            if k == 1:
                                return slice(s0, s0 + 1)
                            return slice(s0, s0 + 2 * k - 1, 2)

                        def chs(ct, kta, ktb, off):
                            """children h, kt pair, as [p, kt, node, b]."""
                            return ct[:, nsl(off), kta:ktb, :].rearrange(
                                "p n kt b -> p kt n b"
                            )

                    g_i = gpool.tile([128, k, 4, BC], dt_g, name="g_i")
                    g_o = gpool.tile([128, k, 4, BC], dt_g, name="g_o")
                    g_u = gpool.tile([128, k, 4, BC], dt_g, name="g_u")
                    if has_l:
                        g_fl = gpool.tile([128, k, 4, BC], dt_g, name="g_fl", bufs=1)
                    if has_r:
                        g_fr = gpool.tile([128, k, 4, BC], dt_g, name="g_fr", bufs=1)
                    have_f = has_l or has_r
                    if have_f:
                        fx_t = gpool.tile([128, k, 4, BC], dt_g, name="fx_t")

                    def h_chain(ps, j, started):
                        """accumulate the h-path of j into ps (fp8 DoubleRow);
                        2-component child h (h8 + residual) runs two passes."""
                        sides = []
                        if has_l and j < 16:
                            sides.append(("L", 0))
                        if has_r and (j < 12 or 16 <= j):
                            sides.append(("R", 1))
                        insts = []
                        for side, off in sides:
                            i0 = WH_IDX[(side, 0, j)]
                            for ct, _ in child_h:
                                insts.append(
                                    (wh8_sb[:, i0 : i0 + 2, :], chs(ct, 0, 2, off))
                                )
                                insts.append(
                                    (wh8_sb[:, i0 + 2 : i0 + 4, :], chs(ct, 2, 4, off))
                                )
                        for m, (w_ap, rhs) in enumerate(insts):
                            nc.tensor.matmul(
                                ps[:],
                                w_ap,
                                rhs,
                                start=(not started and m == 0),
                                stop=(m == len(insts) - 1),
                                perf_mode=DR,
                            )

                    # --- fx group: x @ W_fx, once per chunk ---
                    if have_f:
                        for jf in range(4):
                            ps = pspool.tile([128, k, BC], f32, name="ps")
                            for kt in range(4):
                                nc.tensor.matmul(
                                    ps[:],
                                    wx_ap(kt, 12 + jf),
                                    xt_t[:, :, kt, :],
                                    start=(kt == 0),
                                    stop=(kt == 3),
                                )
                            nc.scalar.activation(
                                out=fx_t[:, :, jf, :], in_=ps[:], func=ACT.Copy
                            )

                    # --- kt cohorts: js {kt, 4+kt, 8+kt, 12+kt, 16+kt}, then
                    # that kt's elementwise. Each kt chain completes
                    # independently, so the next level's matmuls only wait for
                    # the last cohort instead of the whole chunk, and DVE/Act
                    # work overlaps later cohorts' matmuls.
                    if out_c is not None:
                        oc_t, oc_base = out_c
                        c_t = oc_t[:, a - oc_base : b_ - oc_base, :, :]
                    else:
                        c_t = epool.tile([128, k, 4, BC], dt_g, name="c_t")[:]
                    if have_f:
                        acc = epool.tile([128, k, 4, BC], dt_g, name="acc", bufs=1)
                        m2f = epool.tile([128, k, 4, BC], dt_g, name="m2f", bufs=1)
                        if has_l and has_r:
                            m3f = epool.tile([128, k, 4, BC], dt_g, name="m3f", bufs=1)
                        tmpf_l = gpool.tile([128, k, 4, BC], dt_g, name="tmpf_l", bufs=1)
                        tmpf_r = gpool.tile([128, k, 4, BC], dt_g, name="tmpf_r", bufs=1)
                    tc_t = epool.tile([128, k, 4, BC], dt_g, name="tc_t", bufs=1)
                    if to_out:
                        h_t = epool.tile([128, k, 4, BC], dt_g, name="h_t")
                    if out_h is not None and len(out_h) == 2:
                        hbf = epool.tile([128, k, 4, BC], bf16, name="hbf", bufs=1)
                        hsl = slice(a - out_h[0][1], b_ - out_h[0][1])

                    for kt in range(4):
                        cjs = [kt, 4 + kt, 8 + kt]
                        if has_l:
                            cjs.append(12 + kt)
                        if has_r:
                            cjs.append(16 + kt)
                        for j in cjs:
                            ps = pspool.tile([128, k, BC], f32, name="ps")
                            if j < 12:
                                for kk in range(4):
                                    nc.tensor.matmul(
                                        ps[:],
                                        wx_ap(kk, j),
                                        xt_t[:, :, kk, :],
                                        start=(kk == 0),
                                        stop=(kk == 3 and child_h is None),
                                    )
                                if child_h is not None:
                                    h_chain(ps, j, started=True)
                                func = ACT.Tanh if 8 <= j else ACT.Sigmoid
                                dst = (g_i, g_o, g_u)[j // 4][:, :, kt, :]
                                nc.scalar.activation(
                                    out=dst,
                                    in_=ps[:],
                                    func=func,
                                    bias=bias_sb[:, j : j + 1],
                                    scale=1.0 / WSCALE,
                                )
                            else:
                                # f gate: h-path in PSUM + fx_t via DVE
                                h_chain(ps, j, started=False)
                                tmp = tmpf_l if j < 16 else tmpf_r
                                tslice = tmp[:, :, kt, :]
                                nc.vector.tensor_add(
                                    tslice, ps[:], fx_t[:, :, kt, :]
                                )
                                g_f = g_fl if j < 16 else g_fr
                                nc.scalar.activation(
                                    out=g_f[:, :, kt, :],
                                    in_=tslice,
                                    func=ACT.Sigmoid,
                                    bias=bias_sb[:, j : j + 1],
                                    scale=1.0 / WSCALE,
                                )

                        # --- elementwise for this kt ---
                        ct_s = c_t[:, :, kt, :]
                        ei = g_i[:, :, kt, :]
                        eu = g_u[:, :, kt, :]
                        eo = g_o[:, :, kt, :]
                        if not have_f:
                            nc.vector.tensor_mul(ct_s, ei, eu)
                        else:
                            accs = acc[:, :, kt, :]
                            nc.vector.tensor_mul(accs, ei, eu)
                            m2s = m2f[:, :, kt, :]
                            if has_l:
                                nc.vector.tensor_mul(
                                    m2s, g_fl[:, :, kt, :], cl_t[:, :, kt, :]
                                )
                            else:
                                nc.vector.tensor_mul(
                                    m2s, g_fr[:, :, kt, :], cr_t[:, :, kt, :]
                                )
                            if has_l and has_r:
                                nc.vector.tensor_add(accs, accs, m2s)
                                m3s = m3f[:, :, kt, :]
                                nc.vector.tensor_mul(
                                    m3s, g_fr[:, :, kt, :], cr_t[:, :, kt, :]
                                )
                                nc.vector.tensor_add(ct_s, accs, m3s)
                            else:
                                nc.vector.tensor_add(ct_s, accs, m2s)
                        tcs = tc_t[:, :, kt, :]
                        nc.scalar.activation(out=tcs, in_=ct_s, func=ACT.Tanh)
                        if to_out:
                            nc.vector.tensor_mul(h_t[:, :, kt, :], eo, tcs)
                        elif len(out_h) == 1:
                            oh_t, oh_base = out_h[0]
                            nc.vector.tensor_mul(
                                oh_t[:, a - oh_base : b_ - oh_base, kt, :], eo, tcs
                            )
                        else:
                            # split-h storage: h8 = fp8(h), r8 = fp8(h - h8);
                            # two DoubleRow passes at the parent recover ~bf16
                            # precision from fp8-weight matmuls.
                            hbs = hbf[:, :, kt, :]
                            nc.vector.tensor_mul(hbs, eo, tcs)
                            h8s = out_h[0][0][:, hsl, kt, :]
                            nc.vector.tensor_copy(h8s, hbs)
                            nc.vector.tensor_sub(
                                out_h[1][0][:, hsl, kt, :], hbs, h8s
                            )

                    if to_out:
                        nc.sync.dma_start(out=c0t_r[:], in_=c_t[:, 0, :, :])
                        nc.sync.dma_start(out=h0t_r[:], in_=h_t[:, 0, :, :])

            # h storage: plain fp8 for levels 4..7; split fp8 (h8 + residual)
            # for levels 1..3, whose parents need ~bf16 h precision.
            # c lives entirely in SBUF: fp8 at level 7 (bounded |i*u| < 1,
            # 7 forget-gates of attenuation), bf16 below.
            H_SPLIT_LVLS = (3, 2, 1)

            for _rep in range(reps):
                leafc_h = hpool.tile([128, 129, 4, BC], fp8, name="h_leafc")
                leafc_c = hpool.tile([128, 129, 4, BC], fp8, name="c_leafc")
                lvl_h = {7: [(leafc_h, 127)]}
                lvl_c = {7: (leafc_c, 127)}
                for lvl in range(6, 0, -1):
                    base = 2**lvl - 1
                    if lvl in H_SPLIT_LVLS:
                        t8 = hpool.tile([128, 2**lvl, 4, BC], fp8, name=f"h_{lvl}")
                        r8 = hpool.tile([128, 2**lvl, 4, BC], fp8, name=f"hr_{lvl}")
                        lvl_h[lvl] = [(t8, base), (r8, base)]
                    else:
                        t = hpool.tile([128, 2**lvl, 4, BC], fp8, name=f"h_{lvl}")
                        lvl_h[lvl] = [(t, base)]
                    t = hpool.tile([128, 2**lvl, 4, BC], bf16, name=f"c_{lvl}")
                    lvl_c[lvl] = (t, base)

                # leaves: nodes 128..255 (no children)
                process(
                    range(128, 256), False, False, bleaf_sb, None, lvl_h[7],
                    out_c=lvl_c[7],
                )
                # node 127: left child only (node 255, leafc slot 128)
                process(
                    range(127, 128), True, False, b1_sb, lvl_h[7], lvl_h[7],
                    child_c=lvl_c[7], out_c=lvl_c[7],
                )
                # levels 6..1: two children each. L6's first chunk (nodes
                # 63..78) needs node 127's h, so it runs last — the other
                # three L6 chunks (pure-leaf children) hide node 127's serial
                # latency.
                for lvl in range(6, 0, -1):
                    process(
                        range(2**lvl - 1, 2 ** (lvl + 1) - 1),
                        True,
                        True,
                        b2_sb,
                        lvl_h[lvl + 1] if lvl < 6 else lvl_h[7],
                        lvl_h[lvl],
                        child_c=lvl_c[lvl + 1] if lvl < 6 else lvl_c[7],
                        out_c=lvl_c[lvl],
                        chunk_starts=[79, 95, 111, 63] if lvl == 6 else None,
                    )
                # root
                process(
                    range(0, 1), True, True, b2_sb, lvl_h[1], None,
                    child_c=lvl_c[1],
                )

    nc.compile()
    return nc


def _expected_tree():
    left = np.array([2 * i + 1 if 2 * i + 1 < N else 0 for i in range(N)], np.int32)
    right = np.array([2 * i + 2 if 2 * i + 2 < N else 0 for i in range(N)], np.int32)
    nch = np.array(
        [int(2 * i + 1 < N) + int(2 * i + 2 < N) for i in range(N)], np.int32
    )
    return left, right, nch


def pack_w(W_ioux, W_fx, W_iouhL, W_fhL, W_iouhR, W_fhR):
    """Returns (wx bf16 [NWX,128,128], wh8 fp8 [NWH,128,128])."""
    s = WSCALE
    WxT = np.asarray(W_ioux, np.float32).T * s  # [512, 1536]
    WfxT = np.asarray(W_fx, np.float32).T * s  # [512, 512]
    wx = np.empty((NWX, 128, 128), np.float32)
    for i, (kt, j) in enumerate(W_X_BLOCKS):
        src = WxT if j < 12 else WfxT
        jj = j if j < 12 else j - 12
        wx[i] = src[kt * 128 : (kt + 1) * 128, jj * 128 : (jj + 1) * 128]

    WhT = {
        "L": (np.asarray(W_iouhL, np.float32).T * s,
              np.asarray(W_fhL, np.float32).T * s),
        "R": (np.asarray(W_iouhR, np.float32).T * s,
              np.asarray(W_fhR, np.float32).T * s),
    }
    wh = np.empty((NWH, 128, 128), np.float32)
    for i, (side, kt, j) in enumerate(W_H_BLOCKS):
        iou_m, f_m = WhT[side]
        if j < 12:
            wh[i] = iou_m[kt * 128 : (kt + 1) * 128, j * 128 : (j + 1) * 128]
        else:
            jj = (j - 12) if j < 16 else (j - 16)
            wh[i] = f_m[kt * 128 : (kt + 1) * 128, jj * 128 : (jj + 1) * 128]

    return (
        np.ascontiguousarray(wx).astype(BF16),
        np.ascontiguousarray(wh).astype(FP8),
    )


def pack_biases(b_ioux, b_iouh, b_iouhL, b_iouhR, b_fx, b_fhL, b_fhR):
    def pack(vec):
        return np.ascontiguousarray(np.asarray(vec, np.float32).reshape(NJ, 128).T)

    z = np.zeros(512, np.float32)
    b2 = pack(np.concatenate([b_ioux + b_iouhL + b_iouhR, b_fx + b_fhL, b_fx + b_fhR]))
    bleaf = pack(np.concatenate([b_ioux + b_iouh, z, z]))
    b1 = pack(np.concatenate([b_ioux + b_iouhL, b_fx + b_fhL, z]))
    return b2, bleaf, b1


def kernel(
    inputs,
    W_ioux, b_ioux, W_iouh, b_iouh, W_iouhL, b_iouhL, W_iouhR, b_iouhR,
    W_fx, b_fx, W_fh, b_fh, W_fhL, b_fhL, W_fhR, b_fhR,
    left_idx, right_idx, num_children,
):
    el, er, en = _expected_tree()
    assert np.array_equal(np.asarray(left_idx), el), "unexpected tree structure"
    assert np.array_equal(np.asarray(right_idx), er), "unexpected tree structure"
    assert np.array_equal(np.asarray(num_children), en), "unexpected tree structure"

    inputs = np.asarray(inputs, np.float32)

    wx, wh8 = pack_w(W_ioux, W_fx, W_iouhL, W_fhL, W_iouhR, W_fhR)
    b_args = [
        np.asarray(v, np.float32)
        for v in (b_ioux, b_iouh, b_iouhL, b_iouhR, b_fx, b_fhL, b_fhR)
    ]
    b2, bleaf, b1 = pack_biases(*b_args)

    if "nc" not in _compiled:
        _compiled["nc"] = _build_bass()
    nc = _compiled["nc"]

    in_maps = []
    for c in range(NCORES):
        xc = inputs[c * BC : (c + 1) * BC]  # [BC, N, D]
        xt_c = np.ascontiguousarray(xc.transpose(1, 2, 0)).astype(BF16)  # [N, D, BC]
        in_maps.append(
            {"xt": xt_c, "wx": wx, "wh8": wh8,
             "b2": b2, "bleaf": bleaf, "b1": b1}
        )

    res = run_bass_kernel_spmd(
        nc, in_maps, core_ids=list(range(NCORES)), trace=bool(_compiled.get("trace"))
    )
    _compiled["last_res"] = res

    c_full = np.empty((B, D), np.float32)
    h_full = np.empty((B, D), np.float32)
    for c in range(NCORES):
        c_full[c * BC : (c + 1) * BC] = res.results[c]["c0t"].T
        h_full[c * BC : (c + 1) * BC] = res.results[c]["h0t"].T
    return c_full, h_full


# revision 53
# speedup vs baseline: 1.4066x; 1.0010x over previous
"""ConstituencyTreeLSTM Trainium2 kernel.

Strategy:
  - Data-parallel over the B=256 batch across 8 NeuronCores (32 rows/core).
  - The tree is a complete heap (node i has children 2i+1, 2i+2), so the
    sequential scan is reorganized into level-parallel phases:
      leaves (nodes 128..255) -> node 127 -> level 6 (63..126) -> ... -> root.
  - Everything on-device lives in a "feature-on-partitions, (node, ktile,
    batch) on free axis" layout, so matmul outputs (PSUM, [out_dim, rows])
    feed the next level's matmuls with no transposes.
  - h-path matmuls at deep levels (node level >= 3) run in fp8e4m3 with
    DoubleRow perf mode (2 k-tiles per instruction, 2x MAC throughput);
    shallow levels (4+2+1 nodes) stay bf16 for accuracy. x-path matmuls
    are bf16 everywhere (fp8 x fails the error budget). All weights are
    pre-scaled by 16 (exact in bf16, keeps the fp8 h-weights out of the
    e4m3 denormal range); the PSUM-evacuating activation applies
    scale=1/16.
  - The f-gate x-projection (x @ W_fx) is computed once per chunk into
    fx_t (PSUM -> Copy-activation); fL/fR accumulate only their h-path in
    PSUM and a DVE add applies fx_t, removing a duplicated 16-matmul
    group per 2-child chunk.
  - h of every level lives in SBUF level tiles (fp8 for levels 4..7, bf16
    for 1..3); parents read children h via stride-2 node slices
    (rearranged to [p, ktpair, node, batch] for DoubleRow).
  - c goes through DRAM (CL/CR, parity-split by parent index) for the big
    levels; SBUF level tiles for levels 4..1.
  - Weight/bias DMAs ride the Activation HWDGE queue so the first xt tile
    (SP queue) isn't stuck behind them; leaves only wait for the 1.5MB
    iou x-weight tile instead of all weights.
"""

import sys

sys.path.insert(0, "/opt/trn_rl_repo")

import numpy as np
import ml_dtypes

import concourse.bass as bass  # noqa: F401
import concourse.mybir as mybir
import concourse.tile as tile
from concourse import bacc
from concourse.bass_utils import run_bass_kernel_spmd

BF16 = ml_dtypes.bfloat16
FP8 = ml_dtypes.float8_e4m3
NCORES = 8
B, N, D = 256, 256, 512
BC = B // NCORES  # batch rows per core
NJ = 20  # 12 iou + 4 fL + 4 fR bias columns
WSCALE = 16.0

# x-path blocks: 12 iou j-tiles + 4 fx j-tiles, 4 k-tiles each. The iou js
# are ordered by kt-cohort (j = co, 4+co, 8+co) so the first DMA piece covers
# exactly what the first leaf cohort needs.
W_X_BLOCKS = [
    (kt, j) for co in range(4) for j in (co, 4 + co, 8 + co) for kt in range(4)
] + [(kt, j) for j in range(12, 16) for kt in range(4)]
WX_IDX = {p: i for i, p in enumerate(W_X_BLOCKS)}
NWX = len(W_X_BLOCKS)  # 64
NWX_IOU = 48  # iou blocks (cohort-ordered); the rest are the 16 fx blocks

# h-path blocks, DoubleRow-pair adjacent: per iou j: hL kt 0..4 then hR kt
# 0..4; per fL j: hL kt 0..4; per fR j: hR kt 0..4
W_H_BLOCKS = []
for j in range(12):
    W_H_BLOCKS += [("L", kt, j) for kt in range(4)]
    W_H_BLOCKS += [("R", kt, j) for kt in range(4)]
for j in range(12, 16):
    W_H_BLOCKS += [("L", kt, j) for kt in range(4)]
for j in range(16, 20):
    W_H_BLOCKS += [("R", kt, j) for kt in range(4)]
WH_IDX = {p: i for i, p in enumerate(W_H_BLOCKS)}
NWH = len(W_H_BLOCKS)  # 128

_compiled = {}


def _build_bass(reps=1):
    nc = bacc.Bacc("TRN2", target_bir_lowering=False, debug=False, num_devices=NCORES)

    f32 = mybir.dt.float32
    bf16 = mybir.dt.bfloat16
    fp8 = mybir.dt.float8e4
    DR = mybir.MatmulPerfMode.DoubleRow
    ACT = mybir.ActivationFunctionType

    xt = nc.dram_tensor("xt", [N, D, BC], bf16, kind="ExternalInput")
    xt8 = nc.dram_tensor("xt8", [N, D, BC], fp8, kind="ExternalInput")
    wx_d = nc.dram_tensor("wx", [NWX, 128, 128], bf16, kind="ExternalInput")
    wx8_d = nc.dram_tensor("wx8", [NWX, 128, 128], fp8, kind="ExternalInput")
    wh8_d = nc.dram_tensor("wh8", [NWH, 128, 128], fp8, kind="ExternalInput")
    b2_d = nc.dram_tensor("b2", [128, NJ], f32, kind="ExternalInput")
    bleaf_d = nc.dram_tensor("bleaf", [128, NJ], f32, kind="ExternalInput")
    b1_d = nc.dram_tensor("b1", [128, NJ], f32, kind="ExternalInput")

    c0t = nc.dram_tensor("c0t", [D, BC], f32, kind="ExternalOutput")
    h0t = nc.dram_tensor("h0t", [D, BC], f32, kind="ExternalOutput")

    # views: [partition, node, ktile, batch]
    xt_r = xt.ap().rearrange("n (kt p) b -> p n kt b", p=128)
    xt8_r = xt8.ap().rearrange("n (kt p) b -> p n kt b", p=128)
    c0t_r = c0t.ap().rearrange("(kt p) b -> p kt b", p=128)
    h0t_r = h0t.ap().rearrange("(kt p) b -> p kt b", p=128)

    with tile.TileContext(nc) as tc:
        import contextlib

        ctx = contextlib.ExitStack()
        with ctx:
            wpool = ctx.enter_context(tc.tile_pool(name="wpool", bufs=1))
            hpool = ctx.enter_context(tc.tile_pool(name="hpool", bufs=1))
            inpool = ctx.enter_context(tc.tile_pool(name="inpool", bufs=2))
            gpool = ctx.enter_context(tc.tile_pool(name="gpool", bufs=2))
            epool = ctx.enter_context(tc.tile_pool(name="epool", bufs=2))
            pspool = ctx.enter_context(tc.tile_pool(name="ps", bufs=6, space="PSUM"))
            fxpool = ctx.enter_context(tc.tile_pool(name="fxps", bufs=2, space="PSUM"))

            # --- weights / biases ---------------------------------------
            # All weight DMAs ride the Pool (gpsimd) SWDGE queue in 16-block
            # pieces: small pieces interleave with the SP-queue xt prefetches
            # on the DMA engines instead of starving them, and the idle Pool
            # sequencer absorbs the issue cost. The leaf phase only needs the
            # wx_iou pieces (first on the queue) + bleaf (SP, tiny).
            wx_iou_sb = wpool.tile([128, NWX_IOU, 128], bf16, name="wxiou")
            wx_f_sb = wpool.tile([128, NWX - NWX_IOU, 128], bf16, name="wxf")
            wx8_sb = wpool.tile([128, NWX, 128], fp8, name="wx8")
            wh8_sb = wpool.tile([128, NWH, 128], fp8, name="wh8")
            b2_sb = wpool.tile([128, NJ], f32, name="b2sb")
            bleaf_sb = wpool.tile([128, NJ], f32, name="bleafsb")
            b1_sb = wpool.tile([128, NJ], f32, name="b1sb")

            wx_r = wx_d.ap().rearrange("blk p c -> p blk c")
            wx8_r = wx8_d.ap().rearrange("blk p c -> p blk c")
            wh8_r = wh8_d.ap().rearrange("blk p c -> p blk c")
            nc.sync.dma_start(out=bleaf_sb[:], in_=bleaf_d.ap()[:])
            # order: bf16 iou x-blocks (leaves, cohort-piece first), h weights
            # (node 127 / L6), fp8 x-blocks (L6/L5), bf16 fx + biases.
            for s in range(0, NWX_IOU, 12):
                nc.gpsimd.dma_start(
                    out=wx_iou_sb[:, s : s + 12, :], in_=wx_r[:, s : s + 12, :]
                )
            for s in range(0, NWH, 16):
                nc.gpsimd.dma_start(
                    out=wh8_sb[:, s : s + 16, :], in_=wh8_r[:, s : s + 16, :]
                )
            for s in range(0, NWX, 32):
                nc.gpsimd.dma_start(
                    out=wx8_sb[:, s : s + 32, :], in_=wx8_r[:, s : s + 32, :]
                )
            nc.gpsimd.dma_start(out=wx_f_sb[:], in_=wx_r[:, NWX_IOU:, :])
            nc.gpsimd.dma_start(out=b2_sb[:], in_=b2_d.ap()[:])
            nc.gpsimd.dma_start(out=b1_sb[:], in_=b1_d.ap()[:])

            def wx_ap(kt, j):
                if j < 12:
                    return wx_iou_sb[:, WX_IDX[(kt, j)], :]
                return wx_f_sb[:, WX_IDX[(kt, j)] - NWX_IOU, :]

            def process(
                nodes,
                has_l,
                has_r,
                bias_sb,
                child_h,  # list[(tile, base)] — 1 (plain fp8 h) or 2 (h8+res)
                out_h,  # list[(tile, base)] or None (root)
                child_c=None,  # (tile, base_node) -> children c from SBUF
                out_c=None,  # (tile, base_node) -> write c to SBUF
                chunk_starts=None,  # custom chunk order (e.g. L6 defers 63..78)
                x8=False,  # x-path in fp8 DoubleRow (levels 6 and 5)
            ):
                """Compute (c, h) for `nodes` (a range), all at one depth."""
                to_out = out_h is None
                for a in chunk_starts or range(nodes.start, nodes.stop, 16):
                    b_ = min(a + 16, nodes.stop)
                    k = b_ - a  # nodes in this chunk
                    dt_g = f32 if to_out else bf16

                    if x8:
                        xt_t = inpool.tile([128, k, 4, BC], fp8, name="xt8_t")
                        nc.sync.dma_start(out=xt_t[:], in_=xt8_r[:, a:b_, :, :])
                    else:
                        xt_t = inpool.tile([128, k, 4, BC], bf16, name="xt_t")
                        nc.sync.dma_start(out=xt_t[:], in_=xt_r[:, a:b_, :, :])

                    def x_group(ps, j, stop_at_end):
                        """x-path matmuls of j into ps (starts the group)."""
                        if x8:
                            i0 = WX_IDX[(0, j)]
                            for m, kk in enumerate((0, 2)):
                                nc.tensor.matmul(
                                    ps[:],
                                    wx8_sb[:, i0 + kk : i0 + kk + 2, :],
                                    xt_t[:, :, kk : kk + 2, :].rearrange(
                                        "p n kt b -> p kt n b"
                                    ),
                                    start=(m == 0),
                                    stop=(m == 1 and stop_at_end),
                                    perf_mode=DR,
                                )
                        else:
                            for kk in range(4):
                                nc.tensor.matmul(
                                    ps[:],
                                    wx_ap(kk, j),
                                    xt_t[:, :, kk, :],
                                    start=(kk == 0),
                                    stop=(kk == 3 and stop_at_end),
                                )
                    if child_c is not None:
                        cc_t, cc_base = child_c
                        cs0 = 2 * a + 1 - cc_base
                        if has_l:
                            if k == 1:
                                cl_t = cc_t[:, cs0 : cs0 + 1, :, :]
                            else:
                                cl_t = cc_t[:, cs0 : cs0 + 2 * k - 1 : 2, :, :]
                        if has_r:
                            if k == 1:
                                cr_t = cc_t[:, cs0 + 1 : cs0 + 2, :, :]
                            else:
                                cr_t = cc_t[:, cs0 + 1 : cs0 + 2 * k : 2, :, :]

                    if child_h is not None:
                        ch_base = child_h[0][1]
                        sl0 = 2 * a + 1 - ch_base

                        def nsl(off):
                            s0 = sl0 + off
                            if k == 1:
                                return slice(s0, s0 + 1)
                            return slice(s0, s0 + 2 * k - 1, 2)

                        def chs(ct, kta, ktb, off):
                            """children h, kt pair, as [p, kt, node, b]."""
                            return ct[:, nsl(off), kta:ktb, :].rearrange(
                                "p n kt b -> p kt n b"
                            )

                    g_i = gpool.tile([128, k, 4, BC], dt_g, name="g_i")
                    g_o = gpool.tile([128, k, 4, BC], dt_g, name="g_o")
                    g_u = gpool.tile([128, k, 4, BC], dt_g, name="g_u")
                    if has_l:
                        g_fl = gpool.tile([128, k, 4, BC], dt_g, name="g_fl", bufs=1)
                    if has_r:
                        g_fr = gpool.tile([128, k, 4, BC], dt_g, name="g_fr", bufs=1)
                    have_f = has_l or has_r
                    if have_f:
                        fx_t = gpool.tile([128, k, 4, BC], dt_g, name="fx_t")

                    def h_chain(ps, j, started):
                        """accumulate the h-path of j into ps (fp8 DoubleRow);
                        2-component child h (h8 + residual) runs two passes."""
                        sides = []
                        if has_l and j < 16:
                            sides.append(("L", 0))
                        if has_r and (j < 12 or 16 <= j):
                            sides.append(("R", 1))
                        insts = []
                        for side, off in sides:
                            i0 = WH_IDX[(side, 0, j)]
                            for ct, _ in child_h:
                                insts.append(
                                    (wh8_sb[:, i0 : i0 + 2, :], chs(ct, 0, 2, off))
                                )
                                insts.append(
                                    (wh8_sb[:, i0 + 2 : i0 + 4, :], chs(ct, 2, 4, off))
                                )
                        for m, (w_ap, rhs) in enumerate(insts):
                            nc.tensor.matmul(
                                ps[:],
                                w_ap,
                                rhs,
                                start=(not started and m == 0),
                                stop=(m == len(insts) - 1),
                                perf_mode=DR,
                            )

                    # --- kt cohorts: js {kt, 4+kt, 8+kt, 12+kt, 16+kt}, then
                    # that kt's elementwise. Each kt chain completes
                    # independently, so the next level's matmuls only wait for
                    # the last cohort instead of the whole chunk, and DVE/Act
                    # work overlaps later cohorts' matmuls.
                    if out_c is not None:
                        oc_t, oc_base = out_c
                        c_t = oc_t[:, a - oc_base : b_ - oc_base, :, :]
                    else:
                        c_t = epool.tile([128, k, 4, BC], dt_g, name="c_t")[:]
                    if have_f:
                        acc = epool.tile([128, k, 4, BC], dt_g, name="acc", bufs=1)
                        m2f = epool.tile([128, k, 4, BC], dt_g, name="m2f", bufs=1)
                        if has_l and has_r:
                            m3f = epool.tile([128, k, 4, BC], dt_g, name="m3f", bufs=1)
                        tmpf_l = gpool.tile([128, k, 4, BC], dt_g, name="tmpf_l", bufs=1)
                        tmpf_r = gpool.tile([128, k, 4, BC], dt_g, name="tmpf_r", bufs=1)
                    tc_t = epool.tile([128, k, 4, BC], dt_g, name="tc_t", bufs=1)
                    if to_out:
                        h_t = epool.tile([128, k, 4, BC], dt_g, name="h_t")
                    if out_h is not None and len(out_h) == 2:
                        hbf = epool.tile([128, k, 4, BC], bf16, name="hbf", bufs=1)
                        hsl = slice(a - out_h[0][1], b_ - out_h[0][1])

                    for kt in range(4):
                        # this cohort's fx j-tile: PSUM -> SBUF via Act Copy
                        # (walrus forbids a DVE TensorTensor on two PSUM aps)
                        if have_f:
                            ps_fx = fxpool.tile([128, k, BC], f32, name="ps_fx")
                            x_group(ps_fx, 12 + kt, stop_at_end=True)
                            nc.scalar.activation(
                                out=fx_t[:, :, kt, :], in_=ps_fx[:], func=ACT.Copy
                            )
                        cjs = [kt, 4 + kt, 8 + kt]
                        if has_l:
                            cjs.append(12 + kt)
                        if has_r:
                            cjs.append(16 + kt)
                        for j in cjs:
                            ps = pspool.tile([128, k, BC], f32, name="ps")
                            if j < 12:
                                x_group(ps, j, stop_at_end=(child_h is None))
                                if child_h is not None:
                                    h_chain(ps, j, started=True)
                                func = ACT.Tanh if 8 <= j else ACT.Sigmoid
                                dst = (g_i, g_o, g_u)[j // 4][:, :, kt, :]
                                nc.scalar.activation(
                                    out=dst,
                                    in_=ps[:],
                                    func=func,
                                    bias=bias_sb[:, j : j + 1],
                                    scale=1.0 / WSCALE,
                                )
                            else:
                                # f gate: h-path in PSUM + fx psum via DVE
                                h_chain(ps, j, started=False)
                                tmp = tmpf_l if j < 16 else tmpf_r
                                tslice = tmp[:, :, kt, :]
                                nc.vector.tensor_add(
                                    tslice, ps[:], fx_t[:, :, kt, :]
                                )
                                g_f = g_fl if j < 16 else g_fr
                                nc.scalar.activation(
                                    out=g_f[:, :, kt, :],
                                    in_=tslice,
                                    func=ACT.Sigmoid,
                                    bias=bias_sb[:, j : j + 1],
                                    scale=1.0 / WSCALE,
                                )

                        # --- elementwise for this kt ---
                        ct_s = c_t[:, :, kt, :]
                        ei = g_i[:, :, kt, :]
                        eu = g_u[:, :, kt, :]
                        eo = g_o[:, :, kt, :]
                        if not have_f:
                            nc.vector.tensor_mul(ct_s, ei, eu)
                        else:
                            accs = acc[:, :, kt, :]
                            nc.vector.tensor_mul(accs, ei, eu)
                            m2s = m2f[:, :, kt, :]
                            if has_l:
                                nc.vector.tensor_mul(
                                    m2s, g_fl[:, :, kt, :], cl_t[:, :, kt, :]
                                )
                            else:
                                nc.vector.tensor_mul(
                                    m2s, g_fr[:, :, kt, :], cr_t[:, :, kt, :]
                                )
                            if has_l and has_r:
                                nc.vector.tensor_add(accs, accs, m2s)
                                m3s = m3f[:, :, kt, :]
                                nc.vector.tensor_mul(
                                    m3s, g_fr[:, :, kt, :], cr_t[:, :, kt, :]
                                )
                                nc.vector.tensor_add(ct_s, accs, m3s)
                            else:
                                nc.vector.tensor_add(ct_s, accs, m2s)
                        tcs = tc_t[:, :, kt, :]
                        nc.scalar.activation(out=tcs, in_=ct_s, func=ACT.Tanh)
                        if to_out:
                            nc.vector.tensor_mul(h_t[:, :, kt, :], eo, tcs)
                        elif len(out_h) == 1:
                            oh_t, oh_base = out_h[0]
                            nc.vector.tensor_mul(
                                oh_t[:, a - oh_base : b_ - oh_base, kt, :], eo, tcs
                            )
                        else:
                            # split-h storage: h8 = fp8(h), r8 = fp8(h - h8);
                            # two DoubleRow passes at the parent recover ~bf16
                            # precision from fp8-weight matmuls.
                            hbs = hbf[:, :, kt, :]
                            nc.vector.tensor_mul(hbs, eo, tcs)
                            h8s = out_h[0][0][:, hsl, kt, :]
                            nc.vector.tensor_copy(h8s, hbs)
                            nc.vector.tensor_sub(
                                out_h[1][0][:, hsl, kt, :], hbs, h8s
                            )

                    if to_out:
                        nc.sync.dma_start(out=c0t_r[:], in_=c_t[:, 0, :, :])
                        nc.sync.dma_start(out=h0t_r[:], in_=h_t[:, 0, :, :])

            # h storage: plain fp8 for levels 4..7; split fp8 (h8 + residual)
            # for levels 1..3, whose parents need ~bf16 h precision.
            # c lives entirely in SBUF: fp8 at level 7 (bounded |i*u| < 1,
            # 7 forget-gates of attenuation), bf16 below.
            H_SPLIT_LVLS = (3, 2, 1)

            for _rep in range(reps):
                leafc_h = hpool.tile([128, 129, 4, BC], fp8, name="h_leafc")
                leafc_c = hpool.tile([128, 129, 4, BC], fp8, name="c_leafc")
                lvl_h = {7: [(leafc_h, 127)]}
                lvl_c = {7: (leafc_c, 127)}
                for lvl in range(6, 0, -1):
                    base = 2**lvl - 1
                    if lvl in H_SPLIT_LVLS:
                        t8 = hpool.tile([128, 2**lvl, 4, BC], fp8, name=f"h_{lvl}")
                        r8 = hpool.tile([128, 2**lvl, 4, BC], fp8, name=f"hr_{lvl}")
                        lvl_h[lvl] = [(t8, base), (r8, base)]
                    else:
                        t = hpool.tile([128, 2**lvl, 4, BC], fp8, name=f"h_{lvl}")
                        lvl_h[lvl] = [(t, base)]
                    t = hpool.tile([128, 2**lvl, 4, BC], bf16, name=f"c_{lvl}")
                    lvl_c[lvl] = (t, base)

                # leaves: nodes 128..255 (no children)
                process(
                    range(128, 256), False, False, bleaf_sb, None, lvl_h[7],
                    out_c=lvl_c[7],
                )
                # node 127: left child only (node 255, leafc slot 128)
                process(
                    range(127, 128), True, False, b1_sb, lvl_h[7], lvl_h[7],
                    child_c=lvl_c[7], out_c=lvl_c[7],
                )
                # levels 6..1: two children each. L6's first chunk (nodes
                # 63..78) needs node 127's h, so it runs last — the other
                # three L6 chunks (pure-leaf children) hide node 127's serial
                # latency.
                for lvl in range(6, 0, -1):
                    process(
                        range(2**lvl - 1, 2 ** (lvl + 1) - 1),
                        True,
                        True,
                        b2_sb,
                        lvl_h[lvl + 1] if lvl < 6 else lvl_h[7],
                        lvl_h[lvl],
                        child_c=lvl_c[lvl + 1] if lvl < 6 else lvl_c[7],
                        out_c=lvl_c[lvl],
                        chunk_starts=[79, 95, 111, 63] if lvl == 6 else None,
                        x8=(lvl in (6, 5)),
                    )
                # root
                process(
                    range(0, 1), True, True, b2_sb, lvl_h[1], None,
                    child_c=lvl_c[1],
                )

    nc.compile()
    return nc


def _expected_tree():
    left = np.array([2 * i + 1 if 2 * i + 1 < N else 0 for i in range(N)], np.int32)
    right = np.array([2 * i + 2 if 2 * i + 2 < N else 0 for i in range(N)], np.int32)
    nch = np.array(
        [int(2 * i + 1 < N) + int(2 * i + 2 < N) for i in range(N)], np.int32
    )
    return left, right, nch


def pack_w(W_ioux, W_fx, W_iouhL, W_fhL, W_iouhR, W_fhR):
    """Returns (wx bf16, wx8 fp8 [NWX,128,128], wh8 fp8 [NWH,128,128])."""
    s = WSCALE
    WxT = np.asarray(W_ioux, np.float32).T * s  # [512, 1536]
    WfxT = np.asarray(W_fx, np.float32).T * s  # [512, 512]
    wx = np.empty((NWX, 128, 128), np.float32)
    for i, (kt, j) in enumerate(W_X_BLOCKS):
        src = WxT if j < 12 else WfxT
        jj = j if j < 12 else j - 12
        wx[i] = src[kt * 128 : (kt + 1) * 128, jj * 128 : (jj + 1) * 128]

    WhT = {
        "L": (np.asarray(W_iouhL, np.float32).T * s,
              np.asarray(W_fhL, np.float32).T * s),
        "R": (np.asarray(W_iouhR, np.float32).T * s,
              np.asarray(W_fhR, np.float32).T * s),
    }
    wh = np.empty((NWH, 128, 128), np.float32)
    for i, (side, kt, j) in enumerate(W_H_BLOCKS):
        iou_m, f_m = WhT[side]
        if j < 12:
            wh[i] = iou_m[kt * 128 : (kt + 1) * 128, j * 128 : (j + 1) * 128]
        else:
            jj = (j - 12) if j < 16 else (j - 16)
            wh[i] = f_m[kt * 128 : (kt + 1) * 128, jj * 128 : (jj + 1) * 128]

    return (
        np.ascontiguousarray(wx).astype(BF16),
        np.ascontiguousarray(wx).astype(FP8),
        np.ascontiguousarray(wh).astype(FP8),
    )


def pack_biases(b_ioux, b_iouh, b_iouhL, b_iouhR, b_fx, b_fhL, b_fhR):
    def pack(vec):
        return np.ascontiguousarray(np.asarray(vec, np.float32).reshape(NJ, 128).T)

    z = np.zeros(512, np.float32)
    b2 = pack(np.concatenate([b_ioux + b_iouhL + b_iouhR, b_fx + b_fhL, b_fx + b_fhR]))
    bleaf = pack(np.concatenate([b_ioux + b_iouh, z, z]))
    b1 = pack(np.concatenate([b_ioux + b_iouhL, b_fx + b_fhL, z]))
    return b2, bleaf, b1


def kernel(
    inputs,
    W_ioux, b_ioux, W_iouh, b_iouh, W_iouhL, b_iouhL, W_iouhR, b_iouhR,
    W_fx, b_fx, W_fh, b_fh, W_fhL, b_fhL, W_fhR, b_fhR,
    left_idx, right_idx, num_children,
):
    el, er, en = _expected_tree()
    assert np.array_equal(np.asarray(left_idx), el), "unexpected tree structure"
    assert np.array_equal(np.asarray(right_idx), er), "unexpected tree structure"
    assert np.array_equal(np.asarray(num_children), en), "unexpected tree structure"

    inputs = np.asarray(inputs, np.float32)

    wx, wx8, wh8 = pack_w(W_ioux, W_fx, W_iouhL, W_fhL, W_iouhR, W_fhR)
    b_args = [
        np.asarray(v, np.float32)
        for v in (b_ioux, b_iouh, b_iouhL, b_iouhR, b_fx, b_fhL, b_fhR)
    ]
    b2, bleaf, b1 = pack_biases(*b_args)

    if "nc" not in _compiled:
        _compiled["nc"] = _build_bass()
    nc = _compiled["nc"]

    in_maps = []
    for c in range(NCORES):
        xc = inputs[c * BC : (c + 1) * BC]  # [BC, N, D]
        xt_c = np.ascontiguousarray(xc.transpose(1, 2, 0))  # [N, D, BC] f32
        in_maps.append(
            {"xt": xt_c.astype(BF16), "xt8": xt_c.astype(FP8),
             "wx": wx, "wx8": wx8, "wh8": wh8,
             "b2": b2, "bleaf": bleaf, "b1": b1}
        )

    res = run_bass_kernel_spmd(
        nc, in_maps, core_ids=list(range(NCORES)), trace=bool(_compiled.get("trace"))
    )
    _compiled["last_res"] = res

    c_full = np.empty((B, D), np.float32)
    h_full = np.empty((B, D), np.float32)
    for c in range(NCORES):
        c_full[c * BC : (c + 1) * BC] = res.results[c]["c0t"].T
        h_full[c * BC : (c + 1) * BC] = res.results[c]["h0t"].T
    return c_full, h_full
